# revision 7
# baseline (speedup 1.0000x reference)
"""GRUAggregation1d Trainium2 kernel.

Computes, for xs [B=16, 512, L=8192], z_prev [B, 128, L] (all fp32):
    q  = sigmoid(Wq@xs + Uq@z + bq)        (per position l, batch b)
    r  = sigmoid(Wr@xs + Ur@z + br)
    zt = tanh(Wz@xs + Uz@(r*z) + bz)
    out = q*z + (1-q)*zt
Sharding: data-parallel over batch. 8 cores x 2 batches each; weights
replicated. Each core loops over 2 batches x 16 position-tiles of 512.

Per tile: 15 matmuls (3 gates x (4 K-chunks of W + 1 U matmul)) accumulated
in PSUM, sigmoid/tanh on ScalarE (bias fused), gate combine on VectorE.
One-iteration software pipeline: the Uz@(r*z) matmul + tanh + combine of
tile i are emitted during tile i+1 so the PE never stalls on the
r -> r*z dependency chain. The r*z product is emitted BEFORE the previous
tile's combine so it is never queued behind 3 TTs on the Vector FIFO.

Startup: the PE is warmed with ~34 dummy matmuls on a memset tile while
the first DMAs are in flight (the HAM clock gate needs ~3.4us of PE
activity to lift the PE clock from 1.2 to 2.4 GHz), and the first tile's
load is split so the first W chunks land early.

I/O layout is optimized for DMA efficiency and low descriptor-generation
overhead on the Sync HWDGE queue (each dma_start costs ~650ns of queue
time regardless of size):
  - xs and z_prev are packed on the host into ONE bf16 tensor laid out
    [b, tile, 128, 5, 512] (4 xs K-chunks + z), so each tile is a single
    640KB DMA with 5KB-contiguous partition rows.
  - all weights are packed into one [128, 15, 128] bf16 tensor and the
    (W+U) biases into one [128, 3] fp32 tensor: 2 startup DMAs, not 9.
  - output is stored bf16 tile-major ([b, tile, 128, 512]) and unpacked/
    upcast to fp32 on the host; stores are issued on the (otherwise idle)
    GpSimd SWDGE queue so they can never head-of-line-block tile loads on
    the Sync queue (the last store uses the then-idle Sync queue for its
    lower completion latency).
"""

from contextlib import ExitStack

import ml_dtypes
import numpy as np

import concourse.bass as bass
import concourse.mybir as mybir
import concourse.tile as tile
from concourse import bacc
from concourse.bass_utils import run_bass_kernel_spmd

B, IN_DIM, WIDTH, L = 16, 512, 128, 8192
N_CORES = 8
B_PER = B // N_CORES          # batches per core
KC = IN_DIM // 128            # K chunks for the W matmuls
NT = 512                      # positions per tile
N_LT = L // NT                # position tiles per batch
N_WARM = 32                   # PE warm-up dummy matmuls (~3.6us at cold clock)
F32 = mybir.dt.float32
BF16 = mybir.dt.bfloat16

_module_cache = {}


def _build():
    key = ("v4", NT)
    if key in _module_cache:
        return _module_cache[key]

    nc = bacc.Bacc("TRN2", target_bir_lowering=False, debug=False,
                   num_devices=N_CORES)

    xz_d = nc.dram_tensor("xz", [B_PER, N_LT, 128, KC + 1, NT], BF16,
                          kind="ExternalInput").ap()
    wp_d = nc.dram_tensor("wp", [128, 3 * (KC + 1), 128], BF16,
                          kind="ExternalInput").ap()
    bp_d = nc.dram_tensor("bp", [128, 3], F32, kind="ExternalInput").ap()
    out_d = nc.dram_tensor("out", [B_PER, N_LT, 128, NT], BF16,
                           kind="ExternalOutput").ap()

    with tile.TileContext(nc) as tc, ExitStack() as ctx:
        wpool = ctx.enter_context(tc.tile_pool(name="weights", bufs=1))
        io = ctx.enter_context(tc.tile_pool(name="io", bufs=3))
        acts = ctx.enter_context(tc.tile_pool(name="acts", bufs=3))
        psum = ctx.enter_context(tc.tile_pool(name="psum", bufs=2, space="PSUM"))
        wps = ctx.enter_context(tc.tile_pool(name="warm_psum", bufs=1,
                                             space="PSUM"))

        # PE warm-up: dummy matmuls on a zeroed tile, issued while the
        # first data DMAs are still in flight. They have no DMA
        # dependencies, so they start right after the preamble and lift
        # the HAM clock gate (~3.4us of PE busy needed) before the first
        # real matmul. GpSimd's preamble ends earliest, so it does the
        # memset.
        warm = wpool.tile([128, 128], BF16, tag="warm")
        nc.gpsimd.memset(warm[:], 0.0)
        warm_ps = wps.tile([128, 128], F32, tag="warm_ps")
        for _ in range(N_WARM):
            nc.tensor.matmul(warm_ps[:], warm[:], warm[:],
                             start=True, stop=True)

        # Startup loads, finely split so the first gate's operands land as
        # early as possible (each dma_start costs ~650ns of Sync-queue DGE
        # time; data for queued DMAs streams in FIFO order per engine).
        wp = wpool.tile([128, 3 * (KC + 1), 128], BF16, tag="wp")
        bp = wpool.tile([128, 3], F32, tag="bp")
        xz_t0 = io.tile([128, KC + 1, NT], BF16, tag="xz_t")
        nc.sync.dma_start(bp[:], bp_d[:])
        nc.sync.dma_start(wp[:, 0:5, :], wp_d[:, 0:5, :])        # q weights
        nc.sync.dma_start(xz_t0[:, 0:3, :], xz_d[0][0][:, 0:3, :])
        nc.sync.dma_start(xz_t0[:, 3:KC + 1, :], xz_d[0][0][:, 3:KC + 1, :])
        nc.sync.dma_start(wp[:, 5:10, :], wp_d[:, 5:10, :])      # r weights
        nc.sync.dma_start(wp[:, 10:15, :], wp_d[:, 10:15, :])    # z weights

        # one software-pipeline stage of carried state per tile:
        # (zt_psum, rz, q_sbuf, z_slice, out_slice, is_last)
        carry = None

        def finish_uz(carry):
            """Uz matmul of tile i-1, emitted mid-way through tile i so the
            sigmoid_r -> r*z -> Uz chain of i-1 (~1.7us) is covered by
            ~2.1us of independent PE work (zt W of i-1 + q gate + first
            r chunk of i)."""
            zt_ps, rz, q_s, z_sl, out_slice, last = carry
            nc.tensor.matmul(zt_ps[:], wp[:, 2 * 5 + 4, :], rz[:],
                             start=False, stop=True)

        def finish_combine(carry):
            """tanh + out = zt + q*(z - zt) bf16 combine + store of i-1."""
            zt_ps, rz, q_s, z_sl, out_slice, last = carry
            zt_s = acts.tile([128, NT], BF16, tag="zt_s")
            nc.scalar.activation(zt_s[:], zt_ps[:],
                                 mybir.ActivationFunctionType.Tanh,
                                 bias=bp[:, 2:3])
            diff = acts.tile([128, NT], BF16, tag="diff")
            nc.vector.tensor_sub(diff[:], z_sl, zt_s[:])
            prod = acts.tile([128, NT], BF16, tag="prod")
            nc.vector.tensor_mul(prod[:], q_s[:], diff[:])
            o_t = acts.tile([128, NT], BF16, tag="o_t")
            nc.vector.tensor_add(o_t[:], zt_s[:], prod[:])
            eng = nc.sync if last else nc.gpsimd
            eng.dma_start(out_slice, o_t[:])

        for b in range(B_PER):
            for i in range(N_LT):
                if b == 0 and i == 0:
                    xz_t = xz_t0
                else:
                    xz_t = io.tile([128, KC + 1, NT], BF16, tag="xz_t")
                    nc.sync.dma_start(xz_t[:], xz_d[b][i])
                z_sl = xz_t[:, KC, :]

                # ---- q gate ----
                q_ps = psum.tile([128, NT], F32, tag="q_ps")
                for k in range(KC):
                    nc.tensor.matmul(q_ps[:], wp[:, 0 * 5 + k, :],
                                     xz_t[:, k, :],
                                     start=(k == 0), stop=False)
                nc.tensor.matmul(q_ps[:], wp[:, 0 * 5 + 4, :], z_sl,
                                 start=False, stop=True)
                q_s = acts.tile([128, NT], BF16, tag="q_s")
                nc.scalar.activation(q_s[:], q_ps[:],
                                     mybir.ActivationFunctionType.Sigmoid,
                                     bias=bp[:, 0:1])

                # ---- r gate (prev tile's Uz slotted after 1st chunk) ----
                r_ps = psum.tile([128, NT], F32, tag="r_ps")
                nc.tensor.matmul(r_ps[:], wp[:, 1 * 5 + 0, :], xz_t[:, 0, :],
                                 start=True, stop=False)
                if carry is not None:
                    finish_uz(carry)
                for k in range(1, KC):
                    nc.tensor.matmul(r_ps[:], wp[:, 1 * 5 + k, :],
                                     xz_t[:, k, :],
                                     start=False, stop=False)
                nc.tensor.matmul(r_ps[:], wp[:, 1 * 5 + 4, :], z_sl,
                                 start=False, stop=True)
                r_s = acts.tile([128, NT], BF16, tag="r_s")
                nc.scalar.activation(r_s[:], r_ps[:],
                                     mybir.ActivationFunctionType.Sigmoid,
                                     bias=bp[:, 1:2])

                # r*z ahead of the previous tile's combine on the DVE queue
                # (the Uz matmul, slotted mid-tile i+1, needs it)
                rz = acts.tile([128, NT], BF16, tag="rz")
                nc.vector.tensor_mul(rz[:], r_s[:], z_sl)

                if carry is not None:
                    finish_combine(carry)
                    carry = None

                # ---- zt: W part only; Uz@(r*z) lands next iteration ----
                zt_ps = psum.tile([128, NT], F32, tag="zt_ps")
                for k in range(KC):
                    nc.tensor.matmul(zt_ps[:], wp[:, 2 * 5 + k, :],
                                     xz_t[:, k, :],
                                     start=(k == 0), stop=False)

                last = (b == B_PER - 1) and (i == N_LT - 1)
                carry = (zt_ps, rz, q_s, z_sl, out_d[b][i], last)

        finish_uz(carry)
        finish_combine(carry)

    nc.compile()
    _module_cache[key] = nc
    return nc


def _pack_inputs(inputs):
    xs = np.asarray(inputs["xs"], dtype=np.float32)
    zp = np.asarray(inputs["z_prev"], dtype=np.float32)
    assert xs.shape == (B, IN_DIM, L) and zp.shape == (B, WIDTH, L)

    # xz[b, t, p, j, n]: j<KC -> xs[b, j*128+p, t*NT+n]; j=KC -> z[b, p, t*NT+n]
    xz = np.empty((B, N_LT, 128, KC + 1, NT), dtype=ml_dtypes.bfloat16)
    xz[:, :, :, :KC, :] = xs.reshape(B, KC, 128, N_LT, NT).transpose(0, 3, 2, 1, 4)
    xz[:, :, :, KC, :] = zp.reshape(B, 128, N_LT, NT).transpose(0, 2, 1, 3)

    # wp[p, g*5+k, o] = Wg[o, k*128+p]; wp[p, g*5+4, o] = Ug[o, p]
    wp = np.empty((128, 3 * (KC + 1), 128), dtype=ml_dtypes.bfloat16)
    bp = np.empty((128, 3), dtype=np.float32)
    for g, (wn, un, wbn, ubn) in enumerate([
        ("Wq_w", "Uq_w", "Wq_b", "Uq_b"),
        ("Wr_w", "Ur_w", "Wr_b", "Ur_b"),
        ("Wz_w", "Uz_w", "Wz_b", "Uz_b"),
    ]):
        w = np.asarray(inputs[wn], dtype=np.float32)   # [128 out, 512 in]
        u = np.asarray(inputs[un], dtype=np.float32)   # [128 out, 128 in]
        wp[:, g * 5:g * 5 + KC, :] = w.T.reshape(KC, 128, 128).transpose(1, 0, 2)
        wp[:, g * 5 + KC, :] = u.T
        bp[:, g] = (np.asarray(inputs[wbn], dtype=np.float32)
                    + np.asarray(inputs[ubn], dtype=np.float32))
    return xz, np.ascontiguousarray(wp), bp


def _run(inputs, trace=False, **run_kwargs):
    xz, wp, bp = _pack_inputs(inputs)

    nc = _build()
    in_maps = []
    for c in range(N_CORES):
        m = {"xz": np.ascontiguousarray(xz[c * B_PER:(c + 1) * B_PER]),
             "wp": wp, "bp": bp}
        in_maps.append(m)

    res = run_bass_kernel_spmd(nc, in_maps, core_ids=list(range(N_CORES)),
                               trace=trace, **run_kwargs)
    # out [B_PER, N_LT, 128, NT] bf16 -> [B, 128, L] fp32
    out = np.concatenate(
        [np.asarray(res.results[c]["out"]) for c in range(N_CORES)], axis=0)
    out = out.astype(np.float32).transpose(0, 2, 1, 3).reshape(B, WIDTH, L)
    return np.ascontiguousarray(out), res


def kernel(**inputs):
    out, _ = _run(inputs, trace=False)
    return out


# revision 8
# speedup vs baseline: 1.1354x; 1.1354x over previous
"""GRUAggregation1d Trainium2 kernel.

Computes, for xs [B=16, 512, L=8192], z_prev [B, 128, L] (all fp32):
    q  = sigmoid(Wq@xs + Uq@z + bq)        (per position l, batch b)
    r  = sigmoid(Wr@xs + Ur@z + br)
    zt = tanh(Wz@xs + Uz@(r*z) + bz)
    out = q*z + (1-q)*zt
Sharding: data-parallel over batch. 8 cores x 2 batches each; weights
replicated. Each core loops over 2 batches x 16 position-tiles of 512.

Per tile: 15 matmuls (3 gates x (4 K-chunks of W + 1 U matmul)) accumulated
in PSUM, sigmoid/tanh on ScalarE (bias fused), gate combine on VectorE.
One-iteration software pipeline: the Uz@(r*z) matmul + tanh + combine of
tile i are emitted during tile i+1 so the PE never stalls on the
r -> r*z dependency chain. The r*z product is emitted BEFORE the previous
tile's combine so it is never queued behind 3 TTs on the Vector FIFO.

Startup: the PE is warmed with ~34 dummy matmuls on a memset tile while
the first DMAs are in flight (the HAM clock gate needs ~3.4us of PE
activity to lift the PE clock from 1.2 to 2.4 GHz), and the first tile's
load is split so the first W chunks land early.

I/O layout is optimized for DMA efficiency and low descriptor-generation
overhead on the Sync HWDGE queue (each dma_start costs ~650ns of queue
time regardless of size):
  - xs and z_prev are packed on the host into ONE bf16 tensor laid out
    [b, tile, 128, 5, 512] (4 xs K-chunks + z), so each tile is a single
    640KB DMA with 5KB-contiguous partition rows.
  - all weights are packed into one [128, 15, 128] bf16 tensor and the
    (W+U) biases into one [128, 3] fp32 tensor: 2 startup DMAs, not 9.
  - output is stored bf16 tile-major ([b, tile, 128, 512]) and unpacked/
    upcast to fp32 on the host; stores are issued on the (otherwise idle)
    GpSimd SWDGE queue so they can never head-of-line-block tile loads on
    the Sync queue (the last store uses the then-idle Sync queue for its
    lower completion latency).
"""

from contextlib import ExitStack

import ml_dtypes
import numpy as np

import concourse.bass as bass
import concourse.mybir as mybir
import concourse.tile as tile
from concourse import bacc
from concourse.bass_utils import run_bass_kernel_spmd

B, IN_DIM, WIDTH, L = 16, 512, 128, 8192
N_CORES = 8
B_PER = B // N_CORES          # batches per core
KC = IN_DIM // 128            # K chunks for the W matmuls
NT = 512                      # positions per tile
N_LT = L // NT                # position tiles per batch
N_WARM = 32                   # PE warm-up dummy matmuls (~3.6us at cold clock)
F32 = mybir.dt.float32
BF16 = mybir.dt.bfloat16

_module_cache = {}


def _build():
    key = ("v5", NT)
    if key in _module_cache:
        return _module_cache[key]

    nc = bacc.Bacc("TRN2", target_bir_lowering=False, debug=False,
                   num_devices=N_CORES)

    xz_d = nc.dram_tensor("xz", [B_PER, N_LT, 128, KC + 1, NT], BF16,
                          kind="ExternalInput").ap()
    wp_d = nc.dram_tensor("wp", [128, 3 * (KC + 1), 128], BF16,
                          kind="ExternalInput").ap()
    bp_d = nc.dram_tensor("bp", [128, 3], F32, kind="ExternalInput").ap()
    out_d = nc.dram_tensor("out", [B_PER, N_LT, 128, NT], BF16,
                           kind="ExternalOutput").ap()

    with tile.TileContext(nc) as tc, ExitStack() as ctx:
        wpool = ctx.enter_context(tc.tile_pool(name="weights", bufs=1))
        io = ctx.enter_context(tc.tile_pool(name="io", bufs=5))
        acts = ctx.enter_context(tc.tile_pool(name="acts", bufs=3))
        psum = ctx.enter_context(tc.tile_pool(name="psum", bufs=2, space="PSUM"))
        wps = ctx.enter_context(tc.tile_pool(name="warm_psum", bufs=1,
                                             space="PSUM"))

        # PE warm-up: dummy matmuls on a zeroed tile, issued while the
        # first data DMAs are still in flight. They have no DMA
        # dependencies, so they start right after the preamble and lift
        # the HAM clock gate (~3.4us of PE busy needed) before the first
        # real matmul. GpSimd's preamble ends earliest, so it does the
        # memset.
        warm = wpool.tile([128, 128], BF16, tag="warm")
        nc.gpsimd.memset(warm[:], 0.0)
        warm_ps = wps.tile([128, 128], F32, tag="warm_ps")
        for _ in range(N_WARM):
            nc.tensor.matmul(warm_ps[:], warm[:], warm[:],
                             start=True, stop=True)

        # Startup loads, finely split so the first gate's operands land as
        # early as possible (each dma_start costs ~650ns of Sync-queue DGE
        # time; data for queued DMAs streams in FIFO order per engine).
        wp = wpool.tile([128, 3 * (KC + 1), 128], BF16, tag="wp")
        bp = wpool.tile([128, 3], F32, tag="bp")
        xz_t0 = io.tile([128, KC + 1, NT], BF16, tag="xz_t")
        nc.sync.dma_start(bp[:], bp_d[:])
        nc.sync.dma_start(wp[:, 0:5, :], wp_d[:, 0:5, :])        # q weights
        nc.sync.dma_start(xz_t0[:, 0:3, :], xz_d[0][0][:, 0:3, :])
        nc.sync.dma_start(xz_t0[:, 3:KC + 1, :], xz_d[0][0][:, 3:KC + 1, :])
        nc.sync.dma_start(wp[:, 5:10, :], wp_d[:, 5:10, :])      # r weights
        nc.sync.dma_start(wp[:, 10:15, :], wp_d[:, 10:15, :])    # z weights

        # one software-pipeline stage of carried state per tile:
        # (zt_psum, rz, q_sbuf, z_slice, out_slice, is_last)
        carry = None

        def finish_uz(carry):
            """Uz matmul of tile i-1, emitted mid-way through tile i so the
            sigmoid_r -> r*z -> Uz chain of i-1 (~1.7us) is covered by
            ~2.1us of independent PE work (zt W of i-1 + q gate + first
            r chunk of i)."""
            zt_ps, rz, q_s, z_sl, out_slice, last = carry
            nc.tensor.matmul(zt_ps[:], wp[:, 2 * 5 + 4, :], rz[:],
                             start=False, stop=True)

        def finish_combine(carry):
            """tanh + out = zt + q*(z - zt) bf16 combine + store of i-1."""
            zt_ps, rz, q_s, z_sl, out_slice, last = carry
            zt_s = acts.tile([128, NT], BF16, tag="zt_s")
            nc.scalar.activation(zt_s[:], zt_ps[:],
                                 mybir.ActivationFunctionType.Tanh,
                                 bias=bp[:, 2:3])
            diff = acts.tile([128, NT], BF16, tag="diff")
            nc.vector.tensor_sub(diff[:], z_sl, zt_s[:])
            prod = acts.tile([128, NT], BF16, tag="prod")
            nc.vector.tensor_mul(prod[:], q_s[:], diff[:])
            o_t = acts.tile([128, NT], BF16, tag="o_t")
            nc.vector.tensor_add(o_t[:], zt_s[:], prod[:])
            eng = nc.sync if last else nc.gpsimd
            eng.dma_start(out_slice, o_t[:])

        for b in range(B_PER):
            for i in range(N_LT):
                if b == 0 and i == 0:
                    xz_t = xz_t0
                else:
                    xz_t = io.tile([128, KC + 1, NT], BF16, tag="xz_t")
                    nc.sync.dma_start(xz_t[:], xz_d[b][i])
                z_sl = xz_t[:, KC, :]

                # ---- q gate ----
                q_ps = psum.tile([128, NT], F32, tag="q_ps")
                for k in range(KC):
                    nc.tensor.matmul(q_ps[:], wp[:, 0 * 5 + k, :],
                                     xz_t[:, k, :],
                                     start=(k == 0), stop=False)
                nc.tensor.matmul(q_ps[:], wp[:, 0 * 5 + 4, :], z_sl,
                                 start=False, stop=True)
                q_s = acts.tile([128, NT], BF16, tag="q_s")
                nc.scalar.activation(q_s[:], q_ps[:],
                                     mybir.ActivationFunctionType.Sigmoid,
                                     bias=bp[:, 0:1])

                # ---- r gate (prev tile's Uz slotted after 1st chunk) ----
                r_ps = psum.tile([128, NT], F32, tag="r_ps")
                nc.tensor.matmul(r_ps[:], wp[:, 1 * 5 + 0, :], xz_t[:, 0, :],
                                 start=True, stop=False)
                if carry is not None:
                    finish_uz(carry)
                for k in range(1, KC):
                    nc.tensor.matmul(r_ps[:], wp[:, 1 * 5 + k, :],
                                     xz_t[:, k, :],
                                     start=False, stop=False)
                nc.tensor.matmul(r_ps[:], wp[:, 1 * 5 + 4, :], z_sl,
                                 start=False, stop=True)
                r_s = acts.tile([128, NT], BF16, tag="r_s")
                nc.scalar.activation(r_s[:], r_ps[:],
                                     mybir.ActivationFunctionType.Sigmoid,
                                     bias=bp[:, 1:2])

                # r*z ahead of the previous tile's combine on the DVE queue
                # (the Uz matmul, slotted mid-tile i+1, needs it)
                rz = acts.tile([128, NT], BF16, tag="rz")
                nc.vector.tensor_mul(rz[:], r_s[:], z_sl)

                if carry is not None:
                    finish_combine(carry)
                    carry = None

                # ---- zt: W part only; Uz@(r*z) lands next iteration ----
                zt_ps = psum.tile([128, NT], F32, tag="zt_ps")
                for k in range(KC):
                    nc.tensor.matmul(zt_ps[:], wp[:, 2 * 5 + k, :],
                                     xz_t[:, k, :],
                                     start=(k == 0), stop=False)

                last = (b == B_PER - 1) and (i == N_LT - 1)
                carry = (zt_ps, rz, q_s, z_sl, out_d[b][i], last)

        finish_uz(carry)
        finish_combine(carry)

    nc.compile()
    _module_cache[key] = nc
    return nc


def _pack_inputs(inputs):
    xs = np.asarray(inputs["xs"], dtype=np.float32)
    zp = np.asarray(inputs["z_prev"], dtype=np.float32)
    assert xs.shape == (B, IN_DIM, L) and zp.shape == (B, WIDTH, L)

    # xz[b, t, p, j, n]: j<KC -> xs[b, j*128+p, t*NT+n]; j=KC -> z[b, p, t*NT+n]
    xz = np.empty((B, N_LT, 128, KC + 1, NT), dtype=ml_dtypes.bfloat16)
    xz[:, :, :, :KC, :] = xs.reshape(B, KC, 128, N_LT, NT).transpose(0, 3, 2, 1, 4)
    xz[:, :, :, KC, :] = zp.reshape(B, 128, N_LT, NT).transpose(0, 2, 1, 3)

    # wp[p, g*5+k, o] = Wg[o, k*128+p]; wp[p, g*5+4, o] = Ug[o, p]
    wp = np.empty((128, 3 * (KC + 1), 128), dtype=ml_dtypes.bfloat16)
    bp = np.empty((128, 3), dtype=np.float32)
    for g, (wn, un, wbn, ubn) in enumerate([
        ("Wq_w", "Uq_w", "Wq_b", "Uq_b"),
        ("Wr_w", "Ur_w", "Wr_b", "Ur_b"),
        ("Wz_w", "Uz_w", "Wz_b", "Uz_b"),
    ]):
        w = np.asarray(inputs[wn], dtype=np.float32)   # [128 out, 512 in]
        u = np.asarray(inputs[un], dtype=np.float32)   # [128 out, 128 in]
        wp[:, g * 5:g * 5 + KC, :] = w.T.reshape(KC, 128, 128).transpose(1, 0, 2)
        wp[:, g * 5 + KC, :] = u.T
        bp[:, g] = (np.asarray(inputs[wbn], dtype=np.float32)
                    + np.asarray(inputs[ubn], dtype=np.float32))
    return xz, np.ascontiguousarray(wp), bp


def _run(inputs, trace=False, **run_kwargs):
    xz, wp, bp = _pack_inputs(inputs)

    nc = _build()
    in_maps = []
    for c in range(N_CORES):
        m = {"xz": np.ascontiguousarray(xz[c * B_PER:(c + 1) * B_PER]),
             "wp": wp, "bp": bp}
        in_maps.append(m)

    res = run_bass_kernel_spmd(nc, in_maps, core_ids=list(range(N_CORES)),
                               trace=trace, **run_kwargs)
    # out [B_PER, N_LT, 128, NT] bf16 -> [B, 128, L] fp32
    out = np.concatenate(
        [np.asarray(res.results[c]["out"]) for c in range(N_CORES)], axis=0)
    out = out.astype(np.float32).transpose(0, 2, 1, 3).reshape(B, WIDTH, L)
    return np.ascontiguousarray(out), res


def kernel(**inputs):
    out, _ = _run(inputs, trace=False)
    return out
